# revision 1
# baseline (speedup 1.0000x reference)
"""Trainium2 Bass kernel for CustomMHA (B=4, S=2048, D=1024, H=16).

Sharding: 8 cores = 4 batches x 2 head-groups. Core c handles batch c//2,
heads (c%2)*8 .. (c%2)*8+7. Each core computes its heads' QKV projection,
attention, and a partial output projection (its heads' columns of W_o);
the host sums the two partial Y's per batch.

Per-core layout (all matmuls bf16, fp32 PSUM accumulation):
  - x^T [D=1024, S=2048] resident in SBUF; Q^T/K^T computed as
    [dout, token] (dout tile j holds heads 2j, 2j+1 on partition halves),
    V computed as [token, dout] with a ones-column appended per head.
  - scores: S^T[k, q] = K_h^T.T @ Q_h^T per 128-k tile, two heads packed
    into PE row groups (dh=64 contraction at partition base 0 / 64).
  - softmax: no max-subtraction needed (|scores/8| < ~5); exp on ScalarE
    with the 1/sqrt(d_h) folded into the activation scale.
  - AV: lhsT = [V_h | 1] (M=65) so row 64 of the PSUM output accumulates
    the softmax denominator for free; normalize with DVE mul by the
    gpsimd-broadcast reciprocal row.
  - out proj: Y_partial[token, e] accumulated over the core's 512 dims.
"""

import os
import numpy as np
import ml_dtypes

B, S, D, H, DH = 4, 2048, 1024, 16, 64
NCORES = 8
P = 128

_cache = {}


def _build():
    import concourse.bacc as bacc
    import concourse.tile as tile
    from concourse import mybir

    f32 = mybir.dt.float32
    bf16 = mybir.dt.bfloat16
    Exp = mybir.ActivationFunctionType.Exp

    nc = bacc.Bacc("TRN2", target_bir_lowering=False, debug=False)
    xT = nc.dram_tensor("xT", [D, S], bf16, kind="ExternalInput")
    wqk = nc.dram_tensor("wqk", [D, 1024], bf16, kind="ExternalInput")
    wv = nc.dram_tensor("wv", [D, 512], bf16, kind="ExternalInput")
    wo = nc.dram_tensor("wo", [512, D], bf16, kind="ExternalInput")
    y = nc.dram_tensor("y", [S, D], f32, kind="ExternalOutput")

    with tile.TileContext(nc) as tc:
        with tc.tile_pool(name="persist", bufs=1) as persist:
            qt = persist.tile([P, 4, S], bf16)          # Q^T  [dout, tok]
            kt = persist.tile([P, 4, S], bf16)          # K^T  [dout, tok]
            vt = persist.tile([P, 16, 8, 65], bf16)     # V    [tok, head, dh+1]
            ot = persist.tile([P, 4, S], bf16)          # O^T  [dout, tok]
            wo_sb = persist.tile([P, 4, D], bf16)
            nc.sync.dma_start(out=wo_sb[:], in_=wo.ap().rearrange("(c p) e -> p c e", p=P))
            nc.vector.memset(vt[:, :, :, 64:65], 1.0)

            # ---- Phase A: QKV projections ----
            with (tc.tile_pool(name="pha", bufs=1) as pha,
                  tc.tile_pool(name="psA", bufs=4, space="PSUM") as psA):
                x_sb = pha.tile([P, 8, S], bf16)
                wqk_sb = pha.tile([P, 8, 1024], bf16)
                wv_sb = pha.tile([P, 8, 512], bf16)
                nc.sync.dma_start(out=x_sb[:], in_=xT.ap().rearrange("(c p) s -> p c s", p=P))
                nc.sync.dma_start(out=wqk_sb[:], in_=wqk.ap().rearrange("(c p) e -> p c e", p=P))
                nc.sync.dma_start(out=wv_sb[:], in_=wv.ap().rearrange("(c p) e -> p c e", p=P))

                for j in range(8):  # Q dout tiles 0-3, K dout tiles 4-7
                    for tb in range(4):
                        ps = psA.tile([P, 512], f32, tag="ps")
                        for c in range(8):
                            nc.tensor.matmul(
                                ps[:],
                                lhsT=wqk_sb[:, c, j * 128:(j + 1) * 128],
                                rhs=x_sb[:, c, tb * 512:(tb + 1) * 512],
                                start=(c == 0), stop=(c == 7),
                            )
                        dst = qt if j < 4 else kt
                        nc.vector.tensor_copy(
                            dst[:, j % 4, tb * 512:(tb + 1) * 512], ps[:])
                for t in range(16):  # V: [token, dout]
                    ps = psA.tile([P, 512], f32, tag="ps")
                    for c in range(8):
                        nc.tensor.matmul(
                            ps[:],
                            lhsT=x_sb[:, c, t * 128:(t + 1) * 128],
                            rhs=wv_sb[:, c, :],
                            start=(c == 0), stop=(c == 7),
                        )
                    nc.vector.tensor_copy(
                        vt[:, t, :, 0:64],
                        ps[:].rearrange("p (h d) -> p h d", d=64))

            # ---- Phase B: attention (head pairs, q-blocks of 512) ----
            with (tc.tile_pool(name="ptp", bufs=18) as ptp,
                  tc.tile_pool(name="nrm", bufs=4) as nrm,
                  tc.tile_pool(name="otb", bufs=4) as otbp,
                  tc.tile_pool(name="psS", bufs=2, space="PSUM") as psS,
                  tc.tile_pool(name="psO", bufs=2, space="PSUM") as psO):
                for hp in range(4):
                    for qb in range(4):
                        qsl = slice(qb * 512, (qb + 1) * 512)
                        oa = psO.tile([65, 512], f32, tag="oa")
                        ob = psO.tile([65, 512], f32, tag="ob")
                        for kti in range(16):
                            ksl = slice(kti * 128, (kti + 1) * 128)
                            s = psS.tile([P, 1024], f32, tag="s")
                            nc.tensor.matmul(
                                s[:, 0:512],
                                lhsT=kt[0:64, hp, ksl], rhs=qt[0:64, hp, qsl],
                                start=True, stop=True)
                            nc.tensor.matmul(
                                s[:, 512:1024],
                                lhsT=kt[64:128, hp, ksl], rhs=qt[64:128, hp, qsl],
                                start=True, stop=True)
                            pt = ptp.tile([P, 1024], bf16, tag="pt")
                            nc.scalar.activation(pt[:], s[:], Exp, scale=0.125)
                            nc.tensor.matmul(
                                oa[:], lhsT=vt[:, kti, 2 * hp, :],
                                rhs=pt[:, 0:512],
                                start=(kti == 0), stop=(kti == 15))
                            nc.tensor.matmul(
                                ob[:], lhsT=vt[:, kti, 2 * hp + 1, :],
                                rhs=pt[:, 512:1024],
                                start=(kti == 0), stop=(kti == 15))
                        # normalize head a (lanes already aligned: 0-63)
                        rca = nrm.tile([1, 512], f32, tag="rca")
                        nc.vector.reciprocal(rca[:], oa[64:65, :])
                        bca = nrm.tile([64, 512], f32, tag="bca")
                        nc.gpsimd.partition_broadcast(bca[:], rca[:])
                        nc.vector.tensor_mul(ot[0:64, hp, qsl], oa[0:64, :], bca[:])
                        # normalize head b at partitions 0-63, bounce to 64-127
                        rcb = nrm.tile([1, 512], f32, tag="rcb")
                        nc.vector.reciprocal(rcb[:], ob[64:65, :])
                        bcb = nrm.tile([64, 512], f32, tag="bcb")
                        nc.gpsimd.partition_broadcast(bcb[:], rcb[:])
                        otb = otbp.tile([64, 512], bf16, tag="otb")
                        nc.vector.tensor_mul(otb[:], ob[0:64, :], bcb[:])
                        nc.sync.dma_start(out=ot[64:128, hp, qsl], in_=otb[:])

            # ---- Phase C: output projection ----
            with (tc.tile_pool(name="yp", bufs=4) as yp,
                  tc.tile_pool(name="psC", bufs=4, space="PSUM") as psC):
                for t in range(16):
                    for eh in range(2):
                        ps = psC.tile([P, 512], f32, tag="psy")
                        for c in range(4):
                            nc.tensor.matmul(
                                ps[:],
                                lhsT=ot[:, c, t * 128:(t + 1) * 128],
                                rhs=wo_sb[:, c, eh * 512:(eh + 1) * 512],
                                start=(c == 0), stop=(c == 3),
                            )
                        ysb = yp.tile([P, 512], f32, tag="ysb")
                        nc.vector.tensor_copy(ysb[:], ps[:])
                        nc.sync.dma_start(
                            out=y[t * 128:(t + 1) * 128, eh * 512:(eh + 1) * 512],
                            in_=ysb[:])

    nc.compile()
    return nc


def _get_nc():
    if "nc" not in _cache:
        _cache["nc"] = _build()
    return _cache["nc"]


def make_in_maps(x, W_qkv, W_o):
    bf = ml_dtypes.bfloat16
    in_maps = []
    for c in range(NCORES):
        b, g = c // 2, c % 2
        ds = g * 512  # this core's slice of the head-major model dim
        xTc = np.ascontiguousarray(x[b].T.astype(bf))
        wq = W_qkv[ds:ds + 512, :]
        wk = W_qkv[1024 + ds:1024 + ds + 512, :]
        wvc = W_qkv[2048 + ds:2048 + ds + 512, :]
        wqkc = np.ascontiguousarray(
            np.concatenate([wq, wk], axis=0).T.astype(bf))
        wvT = np.ascontiguousarray(wvc.T.astype(bf))
        woT = np.ascontiguousarray(W_o[:, ds:ds + 512].T.astype(bf))
        in_maps.append({"xT": xTc, "wqk": wqkc, "wv": wvT, "wo": woT})
    return in_maps


def kernel(x, W_qkv, W_o):
    from concourse.bass_utils import run_bass_kernel_spmd

    nc = _get_nc()
    in_maps = make_in_maps(np.asarray(x, dtype=np.float32),
                           np.asarray(W_qkv, dtype=np.float32),
                           np.asarray(W_o, dtype=np.float32))
    trace = os.environ.get("KERNEL_TRACE", "") == "1"
    res = run_bass_kernel_spmd(nc, in_maps, core_ids=list(range(NCORES)),
                               trace=trace)
    _cache["last_result"] = res
    Y = np.empty((B, S, D), np.float32)
    for b in range(B):
        Y[b] = res.results[2 * b]["y"] + res.results[2 * b + 1]["y"]
    return Y
